# revision 67
# baseline (speedup 1.0000x reference)
"""GatedCrossAttentionBlock Trainium2 kernel, SPMD over 8 NeuronCores.

Sharding: core c handles batch b=c//2, T1-half h=c%2 (1024 rows of T1).
No collectives. Activations feature-major (transposed); all big matmuls
fp8e4 DoubleRow (2x tensor throughput), accumulating f32 in PSUM.

Scale folding: the whole post-attention residual stream is carried
S2-scaled (S2 a power of two) so Wout/W2 quantization scales cost no
extra ops; host divides the output by S2. LayerNorm mean-subtraction is
folded into the q projection as a rank-1 DoubleRow update; for the FFN
it is an explicit vector add. The additive attention mask (0/-240) is
accumulated into the sim PSUM by an fp8 identity DoubleRow matmul. The
post-attention pipeline (Wout -> LN2 -> FF1 -> FF2) is split per
512-token block so block 0's FFN overlaps block 1's tail.
"""
import sys

for _p in ("/opt/trn_rl_repo", "/root/.axon_site/_ro/trn_rl_repo"):
    if _p not in sys.path:
        sys.path.insert(0, _p)

import numpy as np
import ml_dtypes
from contextlib import ExitStack

import concourse.bass as bass
from concourse import bacc
import concourse.mybir as mybir
import concourse.tile as tile

F32 = mybir.dt.float32
BF16 = mybir.dt.bfloat16
FP8 = mybir.dt.float8e4
AF = mybir.ActivationFunctionType
ALU = mybir.AluOpType
DR = mybir.MatmulPerfMode.DoubleRow

B, T1, TKV, N_, DIM, DL, DH, H, MULT = 4, 2048, 8, 64, 1024, 1024, 64, 8, 4
J = TKV * N_          # 512
INNER = H * DH        # 512
DFF = MULT * DIM      # 4096
TI = 1024             # T1 rows per core
NBLK = 2              # i-blocks of 512 per core
CT = DIM // 128       # 8 c-tiles
TINY = 1e-30
EPS = 1e-5
VP = 72               # padded per-head stride in v_aug (even-aligned DR)

_nc_cache = None
_nc_key = None


def build_nc(SQ, SKV, S1, S2):
    nc = bacc.Bacc()
    d_qoT = nc.declare_dram_parameter("qoT", [DIM, TI], BF16, isOutput=False)
    d_qo8 = nc.declare_dram_parameter("qo8", [128, 8 * TI], FP8,
                                      isOutput=False)
    d_kvq = nc.declare_dram_parameter("kvq", [128, 8 * J], FP8, isOutput=False)
    d_mask = nc.declare_dram_parameter("mask01", [128, 4 * TI], FP8,
                                       isOutput=False)
    d_qm = nc.declare_dram_parameter("qmaskT", [1, TI], BF16, isOutput=False)
    d_wgq = nc.declare_dram_parameter("wgq", [128, 8 * INNER], FP8,
                                      isOutput=False)
    d_cwg = nc.declare_dram_parameter("cwg", [1, 2 * INNER], FP8,
                                      isOutput=False)
    d_wqv = nc.declare_dram_parameter("wqv", [128, 4], F32, isOutput=False)
    d_wkvq = nc.declare_dram_parameter("wkvq", [128, 8 * 2 * INNER], FP8,
                                       isOutput=False)
    d_woq = nc.declare_dram_parameter("woq", [128, 4 * DIM], FP8,
                                      isOutput=False)
    d_id8 = nc.declare_dram_parameter("id8", [128, 512], FP8, isOutput=False)
    d_w1q = nc.declare_dram_parameter("w1q", [128, 8 * DFF], FP8,
                                      isOutput=False)
    d_w1v = nc.declare_dram_parameter("w1v", [128, 32], F32, isOutput=False)
    d_w2q = nc.declare_dram_parameter("w2q", [128, 8 * 32 * 128], FP8,
                                      isOutput=False)
    d_out = nc.declare_dram_parameter("out", [DIM, TI], F32, isOutput=True)

    with tile.TileContext(nc) as tc, ExitStack() as ctx:
        pers = ctx.enter_context(tc.tile_pool(name="pers", bufs=1))
        xT = [pers.tile([128, TI], BF16, tag=f"xT{t}", name=f"xT{t}")
              for t in range(CT)]
        xc8 = [pers.tile([128, 2, TI], FP8, tag=f"xc{t}", name=f"xc{t}")
               for t in range(4)]
        cwg_sb = pers.tile([1, 2, INNER], FP8, tag="cwg", name="cwg_sb")
        id8_sb = pers.tile([128, 2, 256], FP8, tag="id8", name="id8_sb")
        wqv_sb = pers.tile([128, 4], F32, tag="wqv", name="wqv_sb")
        w1v_sb = pers.tile([128, 32], F32, tag="w1v", name="w1v_sb")
        qm_sb = pers.tile([1, TI], BF16, tag="qm", name="qm_sb")
        ones_c = pers.tile([128, 1], BF16, tag="ones_c", name="ones_c")
        ones_r = pers.tile([1, 128], BF16, tag="ones_r", name="ones_r")
        ones8c = pers.tile([128, 2, 32], FP8, tag="ones8c", name="ones8c")
        ones8r = pers.tile([1, 2, 64], FP8, tag="ones8r", name="ones8r")
        eps_t = pers.tile([1, 1], F32, tag="eps_t", name="eps_t")
        nmr1 = pers.tile([1, 2, TI], FP8, tag="nmr1", name="nmr1")
        nc.vector.memset(nmr1[:, 1, :], 0.0)
        nc.vector.memset(ones_c[:], 1.0)
        nc.vector.memset(ones_r[:], 1.0)
        nc.vector.memset(ones8c[:], 1.0)
        nc.vector.memset(ones8r[:, 0, :], 1.0)
        nc.vector.memset(ones8r[:, 1, :], 0.0)
        nc.vector.memset(eps_t[:], EPS)

        scr = ctx.enter_context(tc.tile_pool(name="scr", bufs=3))

        def ln_finish(pa, ps_stat, stat_pairs, rb_sb, tag, bsl=None,
                      nmr=None, nm_sb=None):
            """[1,*] stats chain over the token range bsl (or all TI).
            Stats come from UNSCALED fp8 copies; activations are
            S2-scaled, so the broadcast rstd gets a 1/S2 factor."""
            W = 512 if bsl is not None else TI
            mu_t = pa.tile([1, TI], F32, tag="st_mu", name=f"mu{tag}")
            ex2_t = pa.tile([1, TI], F32, tag="st_ex2", name=f"ex2{tag}")
            mu, ex2 = mu_t[:, 0:W], ex2_t[:, 0:W]
            if bsl is None:
                for b, (mu_s, ex_s) in enumerate(stat_pairs):
                    sl = slice(b * 512, b * 512 + 512)
                    nc.vector.tensor_scalar_mul(mu_t[:, sl], mu_s, 1.0 / DIM)
                    nc.vector.tensor_scalar_mul(ex2_t[:, sl], ex_s, 1.0 / DIM)
            else:
                mu_s, ex_s = stat_pairs[0]
                nc.vector.tensor_scalar_mul(mu, mu_s, 1.0 / DIM)
                nc.vector.tensor_scalar_mul(ex2, ex_s, 1.0 / DIM)
            musq_t = pa.tile([1, TI], F32, tag="st_msq", name=f"msq{tag}")
            nc.vector.tensor_mul(musq_t[:, 0:W], mu, mu)
            var_t = pa.tile([1, TI], F32, tag="st_var", name=f"var{tag}")
            nc.vector.tensor_sub(var_t[:, 0:W], ex2, musq_t[:, 0:W])
            std_t = pa.tile([1, TI], F32, tag="st_msq", name=f"std{tag}")
            nc.scalar.activation(std_t[:, 0:W], var_t[:, 0:W], AF.Sqrt,
                                 bias=eps_t[:])
            r_t = pa.tile([1, TI], F32, tag="st_ex2", name=f"r{tag}")
            nc.vector.reciprocal_approx_fast(r_t[:, 0:W], std_t[:, 0:W])
            r = r_t[:, 0:W]
            r_bf_t = pa.tile([1, TI], BF16, tag="st_rbf", name=f"rbf{tag}")
            nc.vector.tensor_scalar_mul(r_bf_t[:, 0:W], r, 1.0 / S2)
            nmrf_t = pa.tile([1, TI], F32, tag="st_var", name=f"nmrf{tag}")
            nc.vector.tensor_mul(nmrf_t[:, 0:W], mu, r)
            if nmr is not None:
                nc.vector.tensor_scalar_mul(nmr[:, 0, :], nmrf_t[:, 0:W],
                                            -64.0)
            nm_bf_t = None
            if nm_sb is not None:
                nm_bf_t = pa.tile([1, TI], BF16, tag="st_nmb",
                                  name=f"nmbf{tag}")
                nc.vector.tensor_scalar_mul(nm_bf_t[:, 0:W], nmrf_t[:, 0:W],
                                            -1.0)
            rngs = ([(b, slice(b * 512, b * 512 + 512)) for b in range(NBLK)]
                    if bsl is None else [(0, bsl)])
            for k, sl in rngs:
                src_ = slice(k * 512, k * 512 + 512) if bsl is None \
                    else slice(0, 512)
                rb_ps = ps_stat.tile([128, 512], F32, tag="w",
                                     name=f"rbps{tag}{k}", bufs=2)
                nc.tensor.matmul(rb_ps[:], ones_r[:], r_bf_t[:, src_],
                                 start=True, stop=True)
                nc.vector.tensor_copy(rb_sb[:, sl], rb_ps[:])
                if nm_sb is not None:
                    nm_ps = ps_stat.tile([128, 512], F32, tag="w",
                                         name=f"nmps{tag}{k}", bufs=2)
                    nc.tensor.matmul(nm_ps[:], ones_r[:], nm_bf_t[:, src_],
                                     start=True, stop=True)
                    nc.vector.tensor_copy(nm_sb[:, sl], nm_ps[:])

        with tc.tile_pool(name="attn", bufs=1) as pa:
            qoT = [pa.tile([128, TI], BF16, tag=f"qoT{t}", name=f"qoT{t}")
                   for t in range(CT)]
            qo8t = [pa.tile([128, 4, TI], FP8, tag=f"qo8{i}",
                            name=f"qo8t{i}") for i in range(2)]
            kv_sb = pa.tile([128, 8, J], FP8, tag="kv", name="kv_sb")
            mask_sb = pa.tile([128, 4, TI], FP8, tag="mask", name="mask_sb")
            wgq_sb = pa.tile([128, 8, INNER], FP8, tag="wgq", name="wgq_sb")
            wkvq_sb = pa.tile([128, 8, 2 * INNER], FP8, tag="wkvq",
                              name="wkvq_sb")
            woq_sb = pa.tile([128, 4, DIM], FP8, tag="woq", name="woq_sb")
            w1q_sb = pa.tile([128, 8, DFF], FP8, tag="w1q", name="w1q_sb")
            rb1_sb = pa.tile([128, TI], F32, tag="rb1", name="rb1_sb")
            rb2_sb = pa.tile([128, TI], F32, tag="rb1", name="rb2_sb")
            nm2_sb = pa.tile([128, TI], F32, tag="nm2", name="nm2_sb")
            xq8 = [pa.tile([128, 2, TI], FP8, tag=f"xq8{t}", name=f"xq8{t}")
                   for t in range(4)]
            _gt_tags = ["mask", "wgq", "kv", "qo80", "qo81",
                        "gt5", "gt6", "gt7"]
            gT4 = [pa.tile([128, 4, TI], FP8, tag=_gt_tags[u],
                           name=f"gT{u}") for u in range(8)]
            # plane layout: tile g, partition 32m+p, pair-index i holds
            # head 4g+m, dh=32i+p — sim contracts dh as 32 partitions x 2
            # DoubleRow subtiles (weights are column-permuted host-side).
            qT8 = [pa.tile([128, 2, TI], FP8, tag=f"qT{g}", name=f"qT{g}")
                   for g in range(2)]
            kT8 = [pa.tile([128, 2, J], FP8, tag=f"kT{g}", name=f"kT{g}")
                   for g in range(2)]
            v_aug = [pa.tile([128, 2, H, VP], FP8, tag=f"vaug{j}",
                             name=f"vaug{j}") for j in range(2)]
            attn_cat = [pa.tile([128, 2, TI], FP8, tag=f"acat{d}",
                                name=f"acat{d}") for d in range(2)]

            _qo8r = d_qo8.rearrange("p (a t) -> p a t", a=8)
            for _i in range(4):
                nc.sync.dma_start(
                    out=qo8t[_i // 2][:, 2 * (_i % 2):2 * (_i % 2) + 2, :],
                    in_=_qo8r[:, 2 * _i:2 * _i + 2])
            nc.sync.dma_start(out=kv_sb,
                              in_=d_kvq.rearrange("p (a j) -> p a j", a=8))
            nc.sync.dma_start(out=wkvq_sb,
                              in_=d_wkvq.rearrange("p (a n) -> p a n", a=8))
            nc.sync.dma_start(out=wgq_sb,
                              in_=d_wgq.rearrange("p (a n) -> p a n", a=8))
            nc.sync.dma_start(out=mask_sb,
                              in_=d_mask.rearrange("p (a t) -> p a t", a=4))
            nc.sync.dma_start(out=qm_sb, in_=d_qm[:, :])
            nc.sync.dma_start(out=cwg_sb,
                              in_=d_cwg.rearrange("p (a n) -> p a n", a=2))
            nc.sync.dma_start(out=wqv_sb, in_=d_wqv[:, :])
            for t in range(CT):
                nc.sync.dma_start(out=qoT[t],
                                  in_=d_qoT[t * 128:(t + 1) * 128, :])
            nc.sync.dma_start(out=id8_sb,
                              in_=d_id8.rearrange("p (a n) -> p a n", a=2))
            nc.sync.dma_start(out=w1v_sb, in_=d_w1v[:, :])
            nc.sync.dma_start(out=woq_sb,
                              in_=d_woq.rearrange("p (a n) -> p a n", a=4))
            nc.sync.dma_start(out=w1q_sb,
                              in_=d_w1q.rearrange("p (a n) -> p a n", a=8))

            for jp in range(2):
                nc.vector.memset(v_aug[jp][:, :, :, DH:DH + 1], 1.0)
                nc.vector.memset(v_aug[jp][:, :, :, DH + 1:VP], 0.0)

            # ---- LN1 stats (fp8 DR) + k/v projections ----
            with tc.tile_pool(name="psStat", bufs=1, space="PSUM") as psStat, \
                 tc.tile_pool(name="psKV", bufs=2, space="PSUM") as psKV:
                st1 = [psStat.tile([32, 2, 512], F32, tag=f"stat{b}",
                                   name=f"st1{b}") for b in range(NBLK)]
                for tp in range(4):
                    qsl = qo8t[tp // 2][:, 2 * (tp % 2):2 * (tp % 2) + 2, :]
                    sq8 = scr.tile([128, 2, TI], FP8, tag="statsq",
                                   name="statsq", bufs=1)
                    nc.vector.tensor_mul(sq8[:], qsl, qsl)
                    for b in range(NBLK):
                        sl = slice(b * 512, b * 512 + 512)
                        nc.tensor.matmul(st1[b][:, 0, :], ones8c[:],
                                         qsl[:, :, sl],
                                         start=(tp == 0), stop=(tp == 3),
                                         perf_mode=DR)
                        nc.tensor.matmul(st1[b][:, 1, :], ones8c[:],
                                         sq8[:, :, sl],
                                         start=(tp == 0), stop=(tp == 3),
                                         perf_mode=DR)
                for d in range(4):
                    k_ps = psKV.tile([128, 512], F32, tag="kv", name="k_ps")
                    for tp in range(4):
                        nc.tensor.matmul(
                            k_ps[:],
                            wkvq_sb[:, 2 * tp:2 * tp + 2,
                                    d * 128:(d + 1) * 128],
                            kv_sb[:, 2 * tp:2 * tp + 2, :],
                            start=(tp == 0), stop=(tp == 3), perf_mode=DR)
                    nc.scalar.activation(kT8[d // 2][:, d % 2, :], k_ps[:],
                                         AF.Copy, scale=1.0 / SKV)
                for c in range(4):
                    v_ps = psKV.tile([128, 512], F32, tag="kv", name="v_ps")
                    for tp in range(4):
                        nc.tensor.matmul(
                            v_ps[:],
                            kv_sb[:, 2 * tp:2 * tp + 2,
                                  c * 128:(c + 1) * 128],
                            wkvq_sb[:, 2 * tp:2 * tp + 2, INNER:2 * INNER],
                            start=(tp == 0), stop=(tp == 3), perf_mode=DR)
                    nc.vector.tensor_scalar_mul(
                        v_aug[c // 2][:, c % 2, :, 0:DH],
                        v_ps[:].rearrange("p (h d) -> p h d", h=H),
                        1.0 / SKV)
                ln_finish(pa, psStat,
                          [(st1[b][0:1, 0, :], st1[b][0:1, 1, :])
                           for b in range(NBLK)], rb1_sb, "1", nmr=nmr1)
            for t in range(CT):
                nc.vector.tensor_mul(xc8[t // 2][:, t % 2, :],
                                     qoT[t][:], rb1_sb[:])

            # ---- q projection ----
            with tc.tile_pool(name="psQ", bufs=2, space="PSUM") as psQ:
                for d in range(4):
                    q_ps = psQ.tile([128, 2, 512], F32, tag="q", name="q_ps")
                    for b in range(NBLK):
                        sl = slice(b * 512, b * 512 + 512)
                        for tp in range(4):
                            nc.tensor.matmul(
                                q_ps[:, b, :],
                                wgq_sb[:, 2 * tp:2 * tp + 2,
                                       d * 128:(d + 1) * 128],
                                xc8[tp][:, :, sl],
                                start=(tp == 0), stop=False, perf_mode=DR)
                        nc.tensor.matmul(q_ps[:, b, :],
                                         cwg_sb[:, :, d * 128:(d + 1) * 128],
                                         nmr1[:, :, sl],
                                         start=False, stop=True, perf_mode=DR)
                    for b in range(NBLK):
                        sl = slice(b * 512, b * 512 + 512)
                        nc.vector.tensor_scalar(qT8[d // 2][:, d % 2, sl],
                                                q_ps[:, b, :],
                                                1.0 / SQ, wqv_sb[:, d:d + 1],
                                                op0=ALU.mult, op1=ALU.add)

            # ---- attention + software-pipelined Wout/LN2/FFN phases ----
            # Emission order interleaves the two 512-token blocks so the
            # serial LN2 chain of one block hides under the other block's
            # tensor work (engine queues execute in emission order).
            with tc.tile_pool(name="psS", bufs=2, space="PSUM") as psS, \
                 tc.tile_pool(name="psAv", bufs=2, space="PSUM") as psAv, \
                 tc.tile_pool(name="psW", bufs=2, space="PSUM") as psW, \
                 tc.tile_pool(name="wstream", bufs=2) as ws:

                def attn_block(b, ff1_fill=None):
                    sl = slice(b * 512, b * 512 + 512)

                    def stage1(h):
                        # sims + additive mask + exp -> pq (fp8)
                        g, m = h // 4, h % 4
                        pr = slice(32 * m, 32 * m + 32)
                        pqs = []
                        for jp in range(2):
                            pq = scr.tile([128, 2, 512], FP8, tag="pq",
                                          name="pq", bufs=4)
                            for i in range(2):
                                jc = 2 * jp + i
                                s_ps = psS.tile([128, 512], F32, tag="sim",
                                                name="s_ps", bufs=3)
                                if i == 0:
                                    # mask accumulated on the tensor engine
                                    nc.tensor.matmul(
                                        s_ps[:],
                                        kT8[g][pr, :, jc * 128:(jc + 1) * 128],
                                        qT8[g][pr, :, sl],
                                        start=True, stop=False, perf_mode=DR,
                                        tile_position=(32 * m, 0))
                                    nc.tensor.matmul(
                                        s_ps[:],
                                        id8_sb[:, :, 0:128],
                                        mask_sb[:, 2 * jp:2 * jp + 2, sl],
                                        start=False, stop=True, perf_mode=DR)
                                    nc.scalar.activation(pq[:, i, :], s_ps[:],
                                                         AF.Exp)
                                else:
                                    # mask added on the vector engine
                                    nc.tensor.matmul(
                                        s_ps[:],
                                        kT8[g][pr, :, jc * 128:(jc + 1) * 128],
                                        qT8[g][pr, :, sl],
                                        start=True, stop=True, perf_mode=DR,
                                        tile_position=(32 * m, 0))
                                    tf = scr.tile([128, 512], BF16,
                                                  tag="nmtmp", name="tf",
                                                  bufs=3)
                                    nc.vector.tensor_add(
                                        tf[:], s_ps[:],
                                        mask_sb[:, jc, sl])
                                    nc.scalar.activation(pq[:, i, :], tf[:],
                                                         AF.Exp)
                            pqs.append(pq)
                        return pqs

                    def stage2(h, pqs):
                        # attention-weighted V (+ denominator row)
                        av_ps = psAv.tile([VP, 512], F32, tag="av",
                                          name="av_ps", bufs=2)
                        for jp in range(2):
                            nc.tensor.matmul(av_ps[:],
                                             v_aug[jp][:, :, h, :],
                                             pqs[jp][:],
                                             start=(jp == 0), stop=(jp == 1),
                                             perf_mode=DR)
                        s_t = scr.tile([1, 512], F32, tag="s_t", name="s_t",
                                       bufs=2)
                        nc.vector.tensor_scalar(s_t[:], av_ps[DH:DH + 1, :],
                                                TINY, 1.0 / 64.0,
                                                op0=ALU.add, op1=ALU.mult)
                        rec = scr.tile([1, 512], F32, tag="rec", name="rec",
                                       bufs=2)
                        nc.vector.reciprocal_approx_fast(rec[:], s_t[:])
                        r8 = scr.tile([1, 2, 512], FP8, tag="rbf_h",
                                      name="rbf_h", bufs=2)
                        nc.vector.memset(r8[:, 1, :], 0.0)
                        nc.vector.tensor_mul(r8[:, 0, :], rec[:], qm_sb[:, sl])
                        return av_ps, r8

                    def stage3(h, av_ps, r8):
                        # broadcast 1/denom and normalize into attn_cat
                        row = 64 * (h % 2)
                        rb_ps = psAv.tile([64, 512], F32, tag="rb",
                                          name="rb_ps", bufs=1)
                        nc.tensor.matmul(rb_ps[:], ones8r[:], r8[:],
                                         start=True, stop=True, perf_mode=DR)
                        rb2 = scr.tile([64, 512], BF16, tag="rb2",
                                       name="rb2", bufs=2)
                        nc.vector.tensor_scalar_mul(rb2[:], rb_ps[:],
                                                    1.0 / 64.0)
                        nc.vector.tensor_mul(
                            attn_cat[h // 4][row:row + 64, (h // 2) % 2, sl],
                            av_ps[0:DH, :], rb2[:])

                    # 2-stage skew keeps the in-order tensor queue stall-free
                    pend = []  # (h, pqs) awaiting stage2; then (h, av, r8)
                    done2 = []
                    for h in range(H):
                        pqs = stage1(h)
                        if ff1_fill is not None:
                            for f in range(4 * h, 4 * h + 4):
                                ff1_one(ff1_fill, f)
                        if pend:
                            ph, ppqs = pend.pop(0)
                            done2.append((ph,) + stage2(ph, ppqs))
                        if done2 and len(done2) > 1 or (done2 and h == H - 1):
                            dh, av_ps, r8 = done2.pop(0)
                            stage3(dh, av_ps, r8)
                        pend.append((h, pqs))
                    while pend:
                        ph, ppqs = pend.pop(0)
                        done2.append((ph,) + stage2(ph, ppqs))
                    while done2:
                        dh, av_ps, r8 = done2.pop(0)
                        stage3(dh, av_ps, r8)

                def wout_stats(b):
                    sl = slice(b * 512, b * 512 + 512)
                    st2mu = psS.tile([32, 512], F32, tag="sim",
                                     name=f"st2mu{b}", bufs=3)
                    st2ss = psS.tile([32, 512], F32, tag="sim",
                                     name=f"st2ss{b}", bufs=3)
                    for e in range(CT):
                        wo_ps = psW.tile([128, 512], F32, tag="w",
                                         name="wo_ps", bufs=2)
                        for dp in range(2):
                            nc.tensor.matmul(
                                wo_ps[:],
                                woq_sb[:, 2 * dp:2 * dp + 2,
                                       e * 128:(e + 1) * 128],
                                attn_cat[dp][:, :, sl],
                                start=(dp == 0), stop=(dp == 1),
                                perf_mode=DR)
                        nc.vector.tensor_add(xT[e][:, sl], wo_ps[:],
                                             qoT[e][:, sl])
                        nc.vector.tensor_scalar_mul(
                            xq8[e // 2][:, e % 2, sl], xT[e][:, sl],
                            1.0 / S2)
                        if e % 2 == 1:
                            tp = e // 2
                            sq8 = scr.tile([128, 2, 512], FP8,
                                           tag="statsq2", name="statsq2",
                                           bufs=2)
                            nc.vector.tensor_mul(sq8[:], xq8[tp][:, :, sl],
                                                 xq8[tp][:, :, sl])
                            nc.tensor.matmul(st2mu[:], ones8c[:],
                                             xq8[tp][:, :, sl],
                                             start=(tp == 0), stop=(tp == 3),
                                             perf_mode=DR)
                            nc.tensor.matmul(st2ss[:], ones8c[:],
                                             sq8[:],
                                             start=(tp == 0), stop=(tp == 3),
                                             perf_mode=DR)
                    return (st2mu, st2ss)

                def ln2_block(b, st2):
                    sl = slice(b * 512, b * 512 + 512)
                    st2mu, st2ss = st2
                    ln_finish(pa, psW, [(st2mu[0:1, :], st2ss[0:1, :])],
                              rb2_sb, f"2{b}", bsl=sl, nm_sb=nm2_sb)
                    for t in range(CT):
                        tmp = scr.tile([128, 512], BF16, tag="nmtmp",
                                       name="nmtmp", bufs=3)
                        nc.vector.tensor_mul(tmp[:], xT[t][:, sl],
                                             rb2_sb[:, sl])
                        nc.vector.tensor_add(xc8[t // 2][:, t % 2, sl],
                                             tmp[:], nm2_sb[:, sl])

                def ff1_one(b, f):
                    sl = slice(b * 512, b * 512 + 512)
                    h_ps = psW.tile([128, 512], F32, tag="w",
                                    name="h_ps", bufs=2)
                    for tp in range(4):
                        nc.tensor.matmul(
                            h_ps[:],
                            w1q_sb[:, 2 * tp:2 * tp + 2,
                                   f * 128:(f + 1) * 128],
                            xc8[tp][:, :, sl],
                            start=(tp == 0), stop=(tp == 3),
                            perf_mode=DR)
                    nc.scalar.activation(
                        gT4[f // 4][:, f % 4, sl], h_ps[:],
                        AF.Gelu, bias=w1v_sb[:, f:f + 1], scale=1.0 / S1)

                def ff1_block(b):
                    for f in range(32):
                        ff1_one(b, f)

                def ff2_block(b):
                    sl = slice(b * 512, b * 512 + 512)
                    w2r = d_w2q.rearrange("p (e a n) -> p e a n", e=8, a=32)
                    for e in range(CT):
                        w2t = ws.tile([128, 32, 128], FP8, tag="w2s",
                                      name="w2t")
                        nc.sync.dma_start(out=w2t, in_=w2r[:, e])
                        h2_ps = psW.tile([128, 512], F32, tag="w",
                                         name="h2_ps", bufs=2)
                        for fp in range(16):
                            nc.tensor.matmul(
                                h2_ps[:],
                                w2t[:, 2 * fp:2 * fp + 2, :],
                                gT4[fp // 2][:, 2 * (fp % 2):2 * (fp % 2) + 2,
                                             sl],
                                start=(fp == 0), stop=(fp == 15),
                                perf_mode=DR)
                        stg = scr.tile([128, 512], F32, tag="stg",
                                       name="stg", bufs=2)
                        nc.vector.tensor_add(stg[:], h2_ps[:], xT[e][:, sl])
                        nc.sync.dma_start(
                            out=d_out[e * 128:(e + 1) * 128, sl],
                            in_=stg[:])

                attn_block(0)
                st0 = wout_stats(0)
                ln2_block(0, st0)
                attn_block(1, ff1_fill=0)
                st1b = wout_stats(1)
                ff2_block(0)
                ln2_block(1, st1b)
                ff1_block(1)
                ff2_block(1)
    nc.compile()
    return nc


def _pow2floor(x):
    return float(2.0 ** np.floor(np.log2(x)))


def _q8(x, s):
    return np.clip(np.asarray(x, np.float64) * s, -240.0, 240.0).astype(
        ml_dtypes.float8_e4m3)


def _pack_rows(w8):
    """[(a*128+p), n] -> [p, (a n)] packed fp8 array."""
    a = w8.shape[0] // 128
    return np.ascontiguousarray(
        w8.reshape(a, 128, -1).transpose(1, 0, 2).reshape(128, -1))


def _scales(inputs):
    f64 = np.float64
    scale = DH ** (-0.5)
    tanh_a = np.tanh(f64(inputs["attn_gate"][0]))
    tanh_f = np.tanh(f64(inputs["ff_gate"][0]))
    Wg = inputs["ln_g"].astype(f64)[:, None] * inputs["Wq"].astype(f64) * scale
    W1g = inputs["ff_ln_g"].astype(f64)[:, None] * inputs["W1"].astype(f64)
    SQ = _pow2floor(224.0 / np.abs(Wg).max())
    SKV = _pow2floor(224.0 / np.abs(inputs["Wkv"]).max())
    S1 = _pow2floor(224.0 / np.abs(W1g).max())
    S2 = min(_pow2floor(224.0 / np.abs(inputs["Wout"] * tanh_a).max()),
             _pow2floor(224.0 / np.abs(inputs["W2"] * tanh_f).max()))
    return SQ, SKV, S1, S2, Wg, W1g, tanh_a, tanh_f


def _prep_in_maps(inputs, SQ, SKV, S1, S2, Wg, W1g, tanh_a, tanh_f):
    bf = ml_dtypes.bfloat16
    f64 = np.float64
    scale = DH ** (-0.5)
    qo = inputs["qo"]
    kvo = inputs["kvo"]
    attn_mask = inputs["attn_mask"]
    q_mask = inputs["q_mask"]
    kv_mask = inputs["kv_mask"]

    # plane permutation: old col n = h*64+dh -> new col (2g+i)*128+32m+p
    # with h=4g+m, dh=32i+p (sim contracts dh via 32 partitions x 2 DR)
    n = np.arange(INNER)
    h_, dh_ = n // 64, n % 64
    newidx = (2 * (h_ // 4) + dh_ // 32) * 128 + 32 * (h_ % 4) + dh_ % 32
    Wg_p = np.empty_like(Wg)
    Wg_p[:, newidx] = Wg
    wgq = _q8(Wg_p, SQ)
    cwg = wgq.astype(np.float32).sum(axis=0)
    cw8g = np.zeros((1, 2 * INNER), dtype=ml_dtypes.float8_e4m3)
    cw8g[0, :INNER] = _q8(cwg / 64.0, 1.0)
    wqv = (inputs["ln_b"].astype(f64) @ inputs["Wq"].astype(f64) * scale)
    wqv_p = np.empty_like(wqv)
    wqv_p[newidx] = wqv
    Wkv_p = np.array(inputs["Wkv"], dtype=f64)
    Wkv_p[:, newidx] = Wkv_p[:, :INNER].copy()
    wkvq = _q8(Wkv_p, SKV)
    woq = _q8(inputs["Wout"].astype(f64) * tanh_a * S2, 1.0)
    w1q = _q8(W1g, S1)
    w1v = (inputs["ff_ln_b"].astype(f64) @ inputs["W1"].astype(f64))
    # fp8 DR identity pair tile: (I,0) at cols 0:128, (0,I) at cols 128:256
    id8 = np.zeros((128, 2, 256), dtype=np.float64)
    id8[np.arange(128), 0, np.arange(128)] = 1.0
    id8[np.arange(128), 1, 128 + np.arange(128)] = 1.0
    id8 = id8.reshape(128, 512).astype(ml_dtypes.float8_e4m3)
    w2q = _q8(inputs["W2"].astype(f64) * tanh_f * S2, 1.0)
    w2p = np.ascontiguousarray(
        w2q.reshape(32, 128, 8, 128).transpose(1, 2, 0, 3).reshape(128, -1))
    shared = {
        "wgq": _pack_rows(wgq),
        "cwg": cw8g,
        "wqv": np.ascontiguousarray(wqv_p.reshape(4, 128).T,
                                    dtype=np.float32),
        "wkvq": _pack_rows(wkvq),
        "woq": _pack_rows(woq),
        "w1q": _pack_rows(w1q),
        "id8": id8,
        "w1v": np.ascontiguousarray(w1v.reshape(32, 128).T,
                                    dtype=np.float32),
        "w2q": w2p,
    }
    in_maps = []
    for c in range(8):
        b, hf = c // 2, c % 2
        rows = slice(hf * TI, (hf + 1) * TI)
        m = (attn_mask[b, rows, :] & kv_mask[b].reshape(J)[None, :])
        mask01 = _pack_rows(
            np.where(m.T, 0.0, -240.0).astype(ml_dtypes.float8_e4m3))
        kvoT = np.asarray(kvo[b], np.float32).reshape(J, DL).T
        qoTs = qo[b, rows, :].T * np.float32(S2)
        im = dict(shared)
        im["qoT"] = np.ascontiguousarray(qoTs, dtype=bf)
        im["qo8"] = _pack_rows(_q8(qo[b, rows, :].T, 1.0))
        im["kvq"] = _pack_rows(_q8(kvoT, 1.0))
        im["mask01"] = mask01
        im["qmaskT"] = np.ascontiguousarray(q_mask[b, rows][None, :],
                                            dtype=bf)
        in_maps.append(im)
    return in_maps


def kernel(**inputs):
    global _nc_cache, _nc_key
    inputs = {k: np.asarray(v) for k, v in inputs.items()}
    SQ, SKV, S1, S2, Wg, W1g, tanh_a, tanh_f = _scales(inputs)
    in_maps = _prep_in_maps(inputs, SQ, SKV, S1, S2, Wg, W1g, tanh_a, tanh_f)
    key = (SQ, SKV, S1, S2)
    if _nc_cache is None or _nc_key != key:
        _nc_cache = build_nc(SQ, SKV, S1, S2)
        _nc_key = key
    from concourse.bass_utils import run_bass_kernel_spmd
    res = run_bass_kernel_spmd(_nc_cache, in_maps, list(range(8)))
    out = np.empty((B, T1, DIM), dtype=np.float32)
    inv = np.float32(1.0 / S2)
    for c in range(8):
        b, hf = c // 2, c % 2
        out[b, hf * TI:(hf + 1) * TI, :] = res.results[c]["out"].T * inv
    return out


if __name__ == "__main__":
    nc = build_nc(2.0 ** 14, 2.0 ** 11, 2.0 ** 11, 2.0 ** 14)
    print("built ok")


# revision 68
# speedup vs baseline: 1.0273x; 1.0273x over previous
"""GatedCrossAttentionBlock Trainium2 kernel, SPMD over 8 NeuronCores.

Sharding: core c handles batch b=c//2, T1-half h=c%2 (1024 rows of T1).
No collectives. Activations feature-major (transposed); all big matmuls
fp8e4 DoubleRow (2x tensor throughput), accumulating f32 in PSUM.

Scale folding: the whole post-attention residual stream is carried
S2-scaled (S2 a power of two) so Wout/W2 quantization scales cost no
extra ops; host divides the output by S2. LayerNorm mean-subtraction is
folded into the q projection as a rank-1 DoubleRow update; for the FFN
it is an explicit vector add. The additive attention mask (0/-240) is
accumulated into the sim PSUM by an fp8 identity DoubleRow matmul. The
post-attention pipeline (Wout -> LN2 -> FF1 -> FF2) is split per
512-token block so block 0's FFN overlaps block 1's tail.
"""
import sys

for _p in ("/opt/trn_rl_repo", "/root/.axon_site/_ro/trn_rl_repo"):
    if _p not in sys.path:
        sys.path.insert(0, _p)

import numpy as np
import ml_dtypes
from contextlib import ExitStack

import concourse.bass as bass
from concourse import bacc
import concourse.mybir as mybir
import concourse.tile as tile

F32 = mybir.dt.float32
BF16 = mybir.dt.bfloat16
FP8 = mybir.dt.float8e4
AF = mybir.ActivationFunctionType
ALU = mybir.AluOpType
DR = mybir.MatmulPerfMode.DoubleRow

B, T1, TKV, N_, DIM, DL, DH, H, MULT = 4, 2048, 8, 64, 1024, 1024, 64, 8, 4
J = TKV * N_          # 512
INNER = H * DH        # 512
DFF = MULT * DIM      # 4096
TI = 1024             # T1 rows per core
NBLK = 2              # i-blocks of 512 per core
CT = DIM // 128       # 8 c-tiles
TINY = 1e-30
EPS = 1e-5
VP = 72               # padded per-head stride in v_aug (even-aligned DR)

_nc_cache = None
_nc_key = None


def build_nc(SQ, SKV, S1, S2):
    nc = bacc.Bacc()
    d_qoT = nc.declare_dram_parameter("qoT", [DIM, TI], BF16, isOutput=False)
    d_qo8 = nc.declare_dram_parameter("qo8", [128, 8 * TI], FP8,
                                      isOutput=False)
    d_kvq = nc.declare_dram_parameter("kvq", [128, 8 * J], FP8, isOutput=False)
    d_mask = nc.declare_dram_parameter("mask01", [128, 4 * TI], FP8,
                                       isOutput=False)
    d_qm = nc.declare_dram_parameter("qmaskT", [1, TI], BF16, isOutput=False)
    d_wgq = nc.declare_dram_parameter("wgq", [128, 8 * INNER], FP8,
                                      isOutput=False)
    d_cwg = nc.declare_dram_parameter("cwg", [1, 2 * INNER], FP8,
                                      isOutput=False)
    d_wqv = nc.declare_dram_parameter("wqv", [128, 4], F32, isOutput=False)
    d_wkvq = nc.declare_dram_parameter("wkvq", [128, 8 * 2 * INNER], FP8,
                                       isOutput=False)
    d_woq = nc.declare_dram_parameter("woq", [128, 4 * DIM], FP8,
                                      isOutput=False)
    d_id8 = nc.declare_dram_parameter("id8", [128, 512], FP8, isOutput=False)
    d_w1q = nc.declare_dram_parameter("w1q", [128, 8 * DFF], FP8,
                                      isOutput=False)
    d_w1v = nc.declare_dram_parameter("w1v", [128, 32], F32, isOutput=False)
    d_w2q = nc.declare_dram_parameter("w2q", [128, 8 * 32 * 128], FP8,
                                      isOutput=False)
    d_out = nc.declare_dram_parameter("out", [DIM, TI], F32, isOutput=True)

    with tile.TileContext(nc) as tc, ExitStack() as ctx:
        pers = ctx.enter_context(tc.tile_pool(name="pers", bufs=1))
        xT = [pers.tile([128, TI], BF16, tag=f"xT{t}", name=f"xT{t}")
              for t in range(CT)]
        xc8 = [pers.tile([128, 2, TI], FP8, tag=f"xc{t}", name=f"xc{t}")
               for t in range(4)]
        cwg_sb = pers.tile([1, 2, INNER], FP8, tag="cwg", name="cwg_sb")
        id8_sb = pers.tile([128, 2, 256], FP8, tag="id8", name="id8_sb")
        wqv_sb = pers.tile([128, 4], F32, tag="wqv", name="wqv_sb")
        w1v_sb = pers.tile([128, 32], F32, tag="w1v", name="w1v_sb")
        qm_sb = pers.tile([1, TI], BF16, tag="qm", name="qm_sb")
        ones_c = pers.tile([128, 1], BF16, tag="ones_c", name="ones_c")
        ones_r = pers.tile([1, 128], BF16, tag="ones_r", name="ones_r")
        ones8c = pers.tile([128, 2, 32], FP8, tag="ones8c", name="ones8c")
        ones8r = pers.tile([1, 2, 64], FP8, tag="ones8r", name="ones8r")
        eps_t = pers.tile([1, 1], F32, tag="eps_t", name="eps_t")
        nmr1 = pers.tile([1, 2, TI], FP8, tag="nmr1", name="nmr1")
        nc.vector.memset(nmr1[:, 1, :], 0.0)
        nc.vector.memset(ones_c[:], 1.0)
        nc.vector.memset(ones_r[:], 1.0)
        nc.vector.memset(ones8c[:], 1.0)
        nc.vector.memset(ones8r[:, 0, :], 1.0)
        nc.vector.memset(ones8r[:, 1, :], 0.0)
        nc.vector.memset(eps_t[:], EPS)

        scr = ctx.enter_context(tc.tile_pool(name="scr", bufs=3))

        def ln_finish(pa, ps_stat, stat_pairs, rb_sb, tag, bsl=None,
                      nmr=None, nm_sb=None):
            """[1,*] stats chain over the token range bsl (or all TI).
            Stats come from UNSCALED fp8 copies; activations are
            S2-scaled, so the broadcast rstd gets a 1/S2 factor."""
            W = 512 if bsl is not None else TI
            mu_t = pa.tile([1, TI], F32, tag="st_mu", name=f"mu{tag}")
            ex2_t = pa.tile([1, TI], F32, tag="st_ex2", name=f"ex2{tag}")
            mu, ex2 = mu_t[:, 0:W], ex2_t[:, 0:W]
            if bsl is None:
                for b, (mu_s, ex_s) in enumerate(stat_pairs):
                    sl = slice(b * 512, b * 512 + 512)
                    nc.vector.tensor_scalar_mul(mu_t[:, sl], mu_s, 1.0 / DIM)
                    nc.vector.tensor_scalar_mul(ex2_t[:, sl], ex_s, 1.0 / DIM)
            else:
                mu_s, ex_s = stat_pairs[0]
                nc.vector.tensor_scalar_mul(mu, mu_s, 1.0 / DIM)
                nc.vector.tensor_scalar_mul(ex2, ex_s, 1.0 / DIM)
            musq_t = pa.tile([1, TI], F32, tag="st_msq", name=f"msq{tag}")
            nc.vector.tensor_mul(musq_t[:, 0:W], mu, mu)
            var_t = pa.tile([1, TI], F32, tag="st_var", name=f"var{tag}")
            nc.vector.tensor_sub(var_t[:, 0:W], ex2, musq_t[:, 0:W])
            std_t = pa.tile([1, TI], F32, tag="st_msq", name=f"std{tag}")
            nc.scalar.activation(std_t[:, 0:W], var_t[:, 0:W], AF.Sqrt,
                                 bias=eps_t[:])
            r_t = pa.tile([1, TI], F32, tag="st_ex2", name=f"r{tag}")
            nc.vector.reciprocal_approx_fast(r_t[:, 0:W], std_t[:, 0:W])
            r = r_t[:, 0:W]
            r_bf_t = pa.tile([1, TI], BF16, tag="st_rbf", name=f"rbf{tag}")
            nc.vector.tensor_scalar_mul(r_bf_t[:, 0:W], r, 1.0 / S2)
            nmrf_t = pa.tile([1, TI], F32, tag="st_var", name=f"nmrf{tag}")
            nc.vector.tensor_mul(nmrf_t[:, 0:W], mu, r)
            if nmr is not None:
                nc.vector.tensor_scalar_mul(nmr[:, 0, :], nmrf_t[:, 0:W],
                                            -64.0)
            nm_bf_t = None
            if nm_sb is not None:
                nm_bf_t = pa.tile([1, TI], BF16, tag="st_nmb",
                                  name=f"nmbf{tag}")
                nc.vector.tensor_scalar_mul(nm_bf_t[:, 0:W], nmrf_t[:, 0:W],
                                            -1.0)
            rngs = ([(b, slice(b * 512, b * 512 + 512)) for b in range(NBLK)]
                    if bsl is None else [(0, bsl)])
            for k, sl in rngs:
                src_ = slice(k * 512, k * 512 + 512) if bsl is None \
                    else slice(0, 512)
                rb_ps = ps_stat.tile([128, 512], F32, tag="w",
                                     name=f"rbps{tag}{k}", bufs=2)
                nc.tensor.matmul(rb_ps[:], ones_r[:], r_bf_t[:, src_],
                                 start=True, stop=True)
                nc.vector.tensor_copy(rb_sb[:, sl], rb_ps[:])
                if nm_sb is not None:
                    nm_ps = ps_stat.tile([128, 512], F32, tag="w",
                                         name=f"nmps{tag}{k}", bufs=2)
                    nc.tensor.matmul(nm_ps[:], ones_r[:], nm_bf_t[:, src_],
                                     start=True, stop=True)
                    nc.vector.tensor_copy(nm_sb[:, sl], nm_ps[:])

        with tc.tile_pool(name="attn", bufs=1) as pa:
            qoT = [pa.tile([128, TI], BF16, tag=f"qoT{t}", name=f"qoT{t}")
                   for t in range(CT)]
            qo8t = [pa.tile([128, 4, TI], FP8, tag=f"qo8{i}",
                            name=f"qo8t{i}") for i in range(2)]
            kv_sb = pa.tile([128, 8, J], FP8, tag="kv", name="kv_sb")
            mask_sb = pa.tile([128, 4, TI], FP8, tag="mask", name="mask_sb")
            wgq_sb = pa.tile([128, 8, INNER], FP8, tag="wgq", name="wgq_sb")
            wkvq_sb = pa.tile([128, 8, 2 * INNER], FP8, tag="wkvq",
                              name="wkvq_sb")
            woq_sb = pa.tile([128, 4, DIM], FP8, tag="woq", name="woq_sb")
            w1q_sb = pa.tile([128, 8, DFF], FP8, tag="w1q", name="w1q_sb")
            rb1_sb = pa.tile([128, TI], F32, tag="rb1", name="rb1_sb")
            rb2_sb = pa.tile([128, TI], F32, tag="rb1", name="rb2_sb")
            nm2_sb = pa.tile([128, TI], F32, tag="nm2", name="nm2_sb")
            xq8 = [pa.tile([128, 2, TI], FP8, tag=f"xq8{t}", name=f"xq8{t}")
                   for t in range(4)]
            _gt_tags = ["mask", "wgq", "kv", "qo80", "qo81",
                        "gt5", "gt6", "gt7"]
            gT4 = [pa.tile([128, 4, TI], FP8, tag=_gt_tags[u],
                           name=f"gT{u}") for u in range(8)]
            # plane layout: tile g, partition 32m+p, pair-index i holds
            # head 4g+m, dh=32i+p — sim contracts dh as 32 partitions x 2
            # DoubleRow subtiles (weights are column-permuted host-side).
            qT8 = [pa.tile([128, 2, TI], FP8, tag=f"qT{g}", name=f"qT{g}")
                   for g in range(2)]
            kT8 = [pa.tile([128, 2, J], FP8, tag=f"kT{g}", name=f"kT{g}")
                   for g in range(2)]
            v_aug = [pa.tile([128, 2, H, VP], FP8, tag=f"vaug{j}",
                             name=f"vaug{j}") for j in range(2)]
            attn_cat = [pa.tile([128, 2, TI], FP8, tag=f"acat{d}",
                                name=f"acat{d}") for d in range(2)]

            _qo8r = d_qo8.rearrange("p (a t) -> p a t", a=8)
            for _i in range(4):
                nc.sync.dma_start(
                    out=qo8t[_i // 2][:, 2 * (_i % 2):2 * (_i % 2) + 2, :],
                    in_=_qo8r[:, 2 * _i:2 * _i + 2])
            nc.sync.dma_start(out=kv_sb,
                              in_=d_kvq.rearrange("p (a j) -> p a j", a=8))
            nc.sync.dma_start(out=wkvq_sb,
                              in_=d_wkvq.rearrange("p (a n) -> p a n", a=8))
            nc.sync.dma_start(out=wgq_sb,
                              in_=d_wgq.rearrange("p (a n) -> p a n", a=8))
            nc.sync.dma_start(out=mask_sb,
                              in_=d_mask.rearrange("p (a t) -> p a t", a=4))
            nc.sync.dma_start(out=qm_sb, in_=d_qm[:, :])
            nc.sync.dma_start(out=cwg_sb,
                              in_=d_cwg.rearrange("p (a n) -> p a n", a=2))
            nc.sync.dma_start(out=wqv_sb, in_=d_wqv[:, :])
            for t in range(CT):
                nc.sync.dma_start(out=qoT[t],
                                  in_=d_qoT[t * 128:(t + 1) * 128, :])
            nc.sync.dma_start(out=id8_sb,
                              in_=d_id8.rearrange("p (a n) -> p a n", a=2))
            nc.sync.dma_start(out=w1v_sb, in_=d_w1v[:, :])
            nc.sync.dma_start(out=woq_sb,
                              in_=d_woq.rearrange("p (a n) -> p a n", a=4))
            nc.sync.dma_start(out=w1q_sb,
                              in_=d_w1q.rearrange("p (a n) -> p a n", a=8))

            for jp in range(2):
                nc.vector.memset(v_aug[jp][:, :, :, DH:DH + 1], 1.0)
                nc.vector.memset(v_aug[jp][:, :, :, DH + 1:VP], 0.0)

            # ---- LN1 stats (fp8 DR) + k/v projections ----
            with tc.tile_pool(name="psStat", bufs=1, space="PSUM") as psStat, \
                 tc.tile_pool(name="psKV", bufs=2, space="PSUM") as psKV:
                st1 = [psStat.tile([32, 2, 512], F32, tag=f"stat{b}",
                                   name=f"st1{b}") for b in range(NBLK)]
                for tp in range(4):
                    qsl = qo8t[tp // 2][:, 2 * (tp % 2):2 * (tp % 2) + 2, :]
                    sq8 = scr.tile([128, 2, TI], FP8, tag="statsq",
                                   name="statsq", bufs=1)
                    nc.vector.tensor_mul(sq8[:], qsl, qsl)
                    for b in range(NBLK):
                        sl = slice(b * 512, b * 512 + 512)
                        nc.tensor.matmul(st1[b][:, 0, :], ones8c[:],
                                         qsl[:, :, sl],
                                         start=(tp == 0), stop=(tp == 3),
                                         perf_mode=DR)
                        nc.tensor.matmul(st1[b][:, 1, :], ones8c[:],
                                         sq8[:, :, sl],
                                         start=(tp == 0), stop=(tp == 3),
                                         perf_mode=DR)
                for d in range(4):
                    k_ps = psKV.tile([128, 512], F32, tag="kv", name="k_ps")
                    for tp in range(4):
                        nc.tensor.matmul(
                            k_ps[:],
                            wkvq_sb[:, 2 * tp:2 * tp + 2,
                                    d * 128:(d + 1) * 128],
                            kv_sb[:, 2 * tp:2 * tp + 2, :],
                            start=(tp == 0), stop=(tp == 3), perf_mode=DR)
                    nc.scalar.activation(kT8[d // 2][:, d % 2, :], k_ps[:],
                                         AF.Copy, scale=1.0 / SKV)
                for c in range(4):
                    v_ps = psKV.tile([128, 512], F32, tag="kv", name="v_ps")
                    for tp in range(4):
                        nc.tensor.matmul(
                            v_ps[:],
                            kv_sb[:, 2 * tp:2 * tp + 2,
                                  c * 128:(c + 1) * 128],
                            wkvq_sb[:, 2 * tp:2 * tp + 2, INNER:2 * INNER],
                            start=(tp == 0), stop=(tp == 3), perf_mode=DR)
                    nc.vector.tensor_scalar_mul(
                        v_aug[c // 2][:, c % 2, :, 0:DH],
                        v_ps[:].rearrange("p (h d) -> p h d", h=H),
                        1.0 / SKV)
                ln_finish(pa, psStat,
                          [(st1[b][0:1, 0, :], st1[b][0:1, 1, :])
                           for b in range(NBLK)], rb1_sb, "1", nmr=nmr1)
            for t in range(CT):
                nc.vector.tensor_mul(xc8[t // 2][:, t % 2, :],
                                     qoT[t][:], rb1_sb[:])

            # ---- q projection ----
            with tc.tile_pool(name="psQ", bufs=2, space="PSUM") as psQ:
                for d in range(4):
                    q_ps = psQ.tile([128, 2, 512], F32, tag="q", name="q_ps")
                    for b in range(NBLK):
                        sl = slice(b * 512, b * 512 + 512)
                        for tp in range(4):
                            nc.tensor.matmul(
                                q_ps[:, b, :],
                                wgq_sb[:, 2 * tp:2 * tp + 2,
                                       d * 128:(d + 1) * 128],
                                xc8[tp][:, :, sl],
                                start=(tp == 0), stop=False, perf_mode=DR)
                        nc.tensor.matmul(q_ps[:, b, :],
                                         cwg_sb[:, :, d * 128:(d + 1) * 128],
                                         nmr1[:, :, sl],
                                         start=False, stop=True, perf_mode=DR)
                    for b in range(NBLK):
                        sl = slice(b * 512, b * 512 + 512)
                        nc.vector.tensor_scalar(qT8[d // 2][:, d % 2, sl],
                                                q_ps[:, b, :],
                                                1.0 / SQ, wqv_sb[:, d:d + 1],
                                                op0=ALU.mult, op1=ALU.add)

            # ---- attention + software-pipelined Wout/LN2/FFN phases ----
            # Emission order interleaves the two 512-token blocks so the
            # serial LN2 chain of one block hides under the other block's
            # tensor work (engine queues execute in emission order).
            with tc.tile_pool(name="psS", bufs=2, space="PSUM") as psS, \
                 tc.tile_pool(name="psAv", bufs=2, space="PSUM") as psAv, \
                 tc.tile_pool(name="psW", bufs=2, space="PSUM") as psW, \
                 tc.tile_pool(name="wstream", bufs=2) as ws:

                def attn_block(b, ff1_fill=None):
                    sl = slice(b * 512, b * 512 + 512)

                    def stage1(h):
                        # sims + additive mask + exp -> pq (fp8)
                        g, m = h // 4, h % 4
                        pr = slice(32 * m, 32 * m + 32)
                        pqs = []
                        for jp in range(2):
                            pq = scr.tile([128, 2, 512], FP8, tag="pq",
                                          name="pq", bufs=4)
                            for i in range(2):
                                jc = 2 * jp + i
                                s_ps = psS.tile([128, 512], F32, tag="sim",
                                                name="s_ps", bufs=2)
                                if i == 0:
                                    # mask accumulated on the tensor engine
                                    nc.tensor.matmul(
                                        s_ps[:],
                                        kT8[g][pr, :, jc * 128:(jc + 1) * 128],
                                        qT8[g][pr, :, sl],
                                        start=True, stop=False, perf_mode=DR,
                                        tile_position=(32 * m, 0))
                                    nc.tensor.matmul(
                                        s_ps[:],
                                        id8_sb[:, :, 0:128],
                                        mask_sb[:, 2 * jp:2 * jp + 2, sl],
                                        start=False, stop=True, perf_mode=DR)
                                    nc.scalar.activation(pq[:, i, :], s_ps[:],
                                                         AF.Exp)
                                else:
                                    # mask added on the vector engine
                                    nc.tensor.matmul(
                                        s_ps[:],
                                        kT8[g][pr, :, jc * 128:(jc + 1) * 128],
                                        qT8[g][pr, :, sl],
                                        start=True, stop=True, perf_mode=DR,
                                        tile_position=(32 * m, 0))
                                    tf = scr.tile([128, 512], BF16,
                                                  tag="nmtmp", name="tf",
                                                  bufs=3)
                                    nc.vector.tensor_add(
                                        tf[:], s_ps[:],
                                        mask_sb[:, jc, sl])
                                    nc.scalar.activation(pq[:, i, :], tf[:],
                                                         AF.Exp)
                            pqs.append(pq)
                        return pqs

                    def stage2(h, pqs):
                        # attention-weighted V (+ denominator row)
                        av_ps = psAv.tile([VP, 512], F32, tag="av",
                                          name="av_ps", bufs=3)
                        for jp in range(2):
                            nc.tensor.matmul(av_ps[:],
                                             v_aug[jp][:, :, h, :],
                                             pqs[jp][:],
                                             start=(jp == 0), stop=(jp == 1),
                                             perf_mode=DR)
                        s_t = scr.tile([1, 512], F32, tag="s_t", name="s_t",
                                       bufs=2)
                        nc.vector.tensor_scalar(s_t[:], av_ps[DH:DH + 1, :],
                                                TINY, 1.0 / 64.0,
                                                op0=ALU.add, op1=ALU.mult)
                        rec = scr.tile([1, 512], F32, tag="rec", name="rec",
                                       bufs=2)
                        nc.vector.reciprocal_approx_fast(rec[:], s_t[:])
                        r8 = scr.tile([1, 2, 512], FP8, tag="rbf_h",
                                      name="rbf_h", bufs=2)
                        nc.vector.memset(r8[:, 1, :], 0.0)
                        nc.vector.tensor_mul(r8[:, 0, :], rec[:], qm_sb[:, sl])
                        return av_ps, r8

                    def stage3(h, av_ps, r8):
                        # broadcast 1/denom and normalize into attn_cat
                        row = 64 * (h % 2)
                        rb_ps = psAv.tile([64, 512], F32, tag="rb",
                                          name="rb_ps", bufs=1)
                        nc.tensor.matmul(rb_ps[:], ones8r[:], r8[:],
                                         start=True, stop=True, perf_mode=DR)
                        rb2 = scr.tile([64, 512], BF16, tag="rb2",
                                       name="rb2", bufs=2)
                        nc.vector.tensor_scalar_mul(rb2[:], rb_ps[:],
                                                    1.0 / 64.0)
                        nc.vector.tensor_mul(
                            attn_cat[h // 4][row:row + 64, (h // 2) % 2, sl],
                            av_ps[0:DH, :], rb2[:])

                    # 2-stage skew keeps the in-order tensor queue stall-free
                    pend = []  # (h, pqs) awaiting stage2; then (h, av, r8)
                    done2 = []
                    for h in range(H):
                        pqs = stage1(h)
                        if ff1_fill is not None:
                            for f in range(4 * h, 4 * h + 4):
                                ff1_one(ff1_fill, f)
                        if pend:
                            ph, ppqs = pend.pop(0)
                            done2.append((ph,) + stage2(ph, ppqs))
                        if done2 and len(done2) > 1 or (done2 and h == H - 1):
                            dh, av_ps, r8 = done2.pop(0)
                            stage3(dh, av_ps, r8)
                        pend.append((h, pqs))
                    while pend:
                        ph, ppqs = pend.pop(0)
                        done2.append((ph,) + stage2(ph, ppqs))
                    while done2:
                        dh, av_ps, r8 = done2.pop(0)
                        stage3(dh, av_ps, r8)

                def wout_stats(b):
                    sl = slice(b * 512, b * 512 + 512)
                    st2mu = psS.tile([32, 512], F32, tag="sim",
                                     name=f"st2mu{b}", bufs=2)
                    st2ss = psS.tile([32, 512], F32, tag="sim",
                                     name=f"st2ss{b}", bufs=2)
                    for e in range(CT):
                        wo_ps = psW.tile([128, 512], F32, tag="w",
                                         name="wo_ps", bufs=2)
                        for dp in range(2):
                            nc.tensor.matmul(
                                wo_ps[:],
                                woq_sb[:, 2 * dp:2 * dp + 2,
                                       e * 128:(e + 1) * 128],
                                attn_cat[dp][:, :, sl],
                                start=(dp == 0), stop=(dp == 1),
                                perf_mode=DR)
                        nc.vector.tensor_add(xT[e][:, sl], wo_ps[:],
                                             qoT[e][:, sl])
                        nc.vector.tensor_scalar_mul(
                            xq8[e // 2][:, e % 2, sl], xT[e][:, sl],
                            1.0 / S2)
                        if e % 2 == 1:
                            tp = e // 2
                            sq8 = scr.tile([128, 2, 512], FP8,
                                           tag="statsq2", name="statsq2",
                                           bufs=2)
                            nc.vector.tensor_mul(sq8[:], xq8[tp][:, :, sl],
                                                 xq8[tp][:, :, sl])
                            nc.tensor.matmul(st2mu[:], ones8c[:],
                                             xq8[tp][:, :, sl],
                                             start=(tp == 0), stop=(tp == 3),
                                             perf_mode=DR)
                            nc.tensor.matmul(st2ss[:], ones8c[:],
                                             sq8[:],
                                             start=(tp == 0), stop=(tp == 3),
                                             perf_mode=DR)
                    return (st2mu, st2ss)

                def ln2_block(b, st2):
                    sl = slice(b * 512, b * 512 + 512)
                    st2mu, st2ss = st2
                    ln_finish(pa, psW, [(st2mu[0:1, :], st2ss[0:1, :])],
                              rb2_sb, f"2{b}", bsl=sl, nm_sb=nm2_sb)
                    for t in range(CT):
                        tmp = scr.tile([128, 512], BF16, tag="nmtmp",
                                       name="nmtmp", bufs=3)
                        nc.vector.tensor_mul(tmp[:], xT[t][:, sl],
                                             rb2_sb[:, sl])
                        nc.vector.tensor_add(xc8[t // 2][:, t % 2, sl],
                                             tmp[:], nm2_sb[:, sl])

                def ff1_one(b, f):
                    sl = slice(b * 512, b * 512 + 512)
                    h_ps = psW.tile([128, 512], F32, tag="w",
                                    name="h_ps", bufs=2)
                    for tp in range(4):
                        nc.tensor.matmul(
                            h_ps[:],
                            w1q_sb[:, 2 * tp:2 * tp + 2,
                                   f * 128:(f + 1) * 128],
                            xc8[tp][:, :, sl],
                            start=(tp == 0), stop=(tp == 3),
                            perf_mode=DR)
                    nc.scalar.activation(
                        gT4[f // 4][:, f % 4, sl], h_ps[:],
                        AF.Gelu, bias=w1v_sb[:, f:f + 1], scale=1.0 / S1)

                def ff1_block(b):
                    for f in range(32):
                        ff1_one(b, f)

                def ff2_block(b):
                    sl = slice(b * 512, b * 512 + 512)
                    w2r = d_w2q.rearrange("p (e a n) -> p e a n", e=8, a=32)
                    for e in range(CT):
                        w2t = ws.tile([128, 32, 128], FP8, tag="w2s",
                                      name="w2t")
                        nc.sync.dma_start(out=w2t, in_=w2r[:, e])
                        h2_ps = psW.tile([128, 512], F32, tag="w",
                                         name="h2_ps", bufs=2)
                        for fp in range(16):
                            nc.tensor.matmul(
                                h2_ps[:],
                                w2t[:, 2 * fp:2 * fp + 2, :],
                                gT4[fp // 2][:, 2 * (fp % 2):2 * (fp % 2) + 2,
                                             sl],
                                start=(fp == 0), stop=(fp == 15),
                                perf_mode=DR)
                        stg = scr.tile([128, 512], F32, tag="stg",
                                       name="stg", bufs=2)
                        nc.vector.tensor_add(stg[:], h2_ps[:], xT[e][:, sl])
                        nc.sync.dma_start(
                            out=d_out[e * 128:(e + 1) * 128, sl],
                            in_=stg[:])

                attn_block(0)
                st0 = wout_stats(0)
                ln2_block(0, st0)
                attn_block(1, ff1_fill=0)
                st1b = wout_stats(1)
                ff2_block(0)
                ln2_block(1, st1b)
                ff1_block(1)
                ff2_block(1)
    nc.compile()
    return nc


def _pow2floor(x):
    return float(2.0 ** np.floor(np.log2(x)))


def _q8(x, s):
    return np.clip(np.asarray(x, np.float64) * s, -240.0, 240.0).astype(
        ml_dtypes.float8_e4m3)


def _pack_rows(w8):
    """[(a*128+p), n] -> [p, (a n)] packed fp8 array."""
    a = w8.shape[0] // 128
    return np.ascontiguousarray(
        w8.reshape(a, 128, -1).transpose(1, 0, 2).reshape(128, -1))


def _scales(inputs):
    f64 = np.float64
    scale = DH ** (-0.5)
    tanh_a = np.tanh(f64(inputs["attn_gate"][0]))
    tanh_f = np.tanh(f64(inputs["ff_gate"][0]))
    Wg = inputs["ln_g"].astype(f64)[:, None] * inputs["Wq"].astype(f64) * scale
    W1g = inputs["ff_ln_g"].astype(f64)[:, None] * inputs["W1"].astype(f64)
    SQ = _pow2floor(224.0 / np.abs(Wg).max())
    SKV = _pow2floor(224.0 / np.abs(inputs["Wkv"]).max())
    S1 = _pow2floor(224.0 / np.abs(W1g).max())
    S2 = min(_pow2floor(224.0 / np.abs(inputs["Wout"] * tanh_a).max()),
             _pow2floor(224.0 / np.abs(inputs["W2"] * tanh_f).max()))
    return SQ, SKV, S1, S2, Wg, W1g, tanh_a, tanh_f


def _prep_in_maps(inputs, SQ, SKV, S1, S2, Wg, W1g, tanh_a, tanh_f):
    bf = ml_dtypes.bfloat16
    f64 = np.float64
    scale = DH ** (-0.5)
    qo = inputs["qo"]
    kvo = inputs["kvo"]
    attn_mask = inputs["attn_mask"]
    q_mask = inputs["q_mask"]
    kv_mask = inputs["kv_mask"]

    # plane permutation: old col n = h*64+dh -> new col (2g+i)*128+32m+p
    # with h=4g+m, dh=32i+p (sim contracts dh via 32 partitions x 2 DR)
    n = np.arange(INNER)
    h_, dh_ = n // 64, n % 64
    newidx = (2 * (h_ // 4) + dh_ // 32) * 128 + 32 * (h_ % 4) + dh_ % 32
    Wg_p = np.empty_like(Wg)
    Wg_p[:, newidx] = Wg
    wgq = _q8(Wg_p, SQ)
    cwg = wgq.astype(np.float32).sum(axis=0)
    cw8g = np.zeros((1, 2 * INNER), dtype=ml_dtypes.float8_e4m3)
    cw8g[0, :INNER] = _q8(cwg / 64.0, 1.0)
    wqv = (inputs["ln_b"].astype(f64) @ inputs["Wq"].astype(f64) * scale)
    wqv_p = np.empty_like(wqv)
    wqv_p[newidx] = wqv
    Wkv_p = np.array(inputs["Wkv"], dtype=f64)
    Wkv_p[:, newidx] = Wkv_p[:, :INNER].copy()
    wkvq = _q8(Wkv_p, SKV)
    woq = _q8(inputs["Wout"].astype(f64) * tanh_a * S2, 1.0)
    w1q = _q8(W1g, S1)
    w1v = (inputs["ff_ln_b"].astype(f64) @ inputs["W1"].astype(f64))
    # fp8 DR identity pair tile: (I,0) at cols 0:128, (0,I) at cols 128:256
    id8 = np.zeros((128, 2, 256), dtype=np.float64)
    id8[np.arange(128), 0, np.arange(128)] = 1.0
    id8[np.arange(128), 1, 128 + np.arange(128)] = 1.0
    id8 = id8.reshape(128, 512).astype(ml_dtypes.float8_e4m3)
    w2q = _q8(inputs["W2"].astype(f64) * tanh_f * S2, 1.0)
    w2p = np.ascontiguousarray(
        w2q.reshape(32, 128, 8, 128).transpose(1, 2, 0, 3).reshape(128, -1))
    shared = {
        "wgq": _pack_rows(wgq),
        "cwg": cw8g,
        "wqv": np.ascontiguousarray(wqv_p.reshape(4, 128).T,
                                    dtype=np.float32),
        "wkvq": _pack_rows(wkvq),
        "woq": _pack_rows(woq),
        "w1q": _pack_rows(w1q),
        "id8": id8,
        "w1v": np.ascontiguousarray(w1v.reshape(32, 128).T,
                                    dtype=np.float32),
        "w2q": w2p,
    }
    in_maps = []
    for c in range(8):
        b, hf = c // 2, c % 2
        rows = slice(hf * TI, (hf + 1) * TI)
        m = (attn_mask[b, rows, :] & kv_mask[b].reshape(J)[None, :])
        mask01 = _pack_rows(
            np.where(m.T, 0.0, -240.0).astype(ml_dtypes.float8_e4m3))
        kvoT = np.asarray(kvo[b], np.float32).reshape(J, DL).T
        qoTs = qo[b, rows, :].T * np.float32(S2)
        im = dict(shared)
        im["qoT"] = np.ascontiguousarray(qoTs, dtype=bf)
        im["qo8"] = _pack_rows(_q8(qo[b, rows, :].T, 1.0))
        im["kvq"] = _pack_rows(_q8(kvoT, 1.0))
        im["mask01"] = mask01
        im["qmaskT"] = np.ascontiguousarray(q_mask[b, rows][None, :],
                                            dtype=bf)
        in_maps.append(im)
    return in_maps


def kernel(**inputs):
    global _nc_cache, _nc_key
    inputs = {k: np.asarray(v) for k, v in inputs.items()}
    SQ, SKV, S1, S2, Wg, W1g, tanh_a, tanh_f = _scales(inputs)
    in_maps = _prep_in_maps(inputs, SQ, SKV, S1, S2, Wg, W1g, tanh_a, tanh_f)
    key = (SQ, SKV, S1, S2)
    if _nc_cache is None or _nc_key != key:
        _nc_cache = build_nc(SQ, SKV, S1, S2)
        _nc_key = key
    from concourse.bass_utils import run_bass_kernel_spmd
    res = run_bass_kernel_spmd(_nc_cache, in_maps, list(range(8)))
    out = np.empty((B, T1, DIM), dtype=np.float32)
    inv = np.float32(1.0 / S2)
    for c in range(8):
        b, hf = c // 2, c % 2
        out[b, hf * TI:(hf + 1) * TI, :] = res.results[c]["out"].T * inv
    return out


if __name__ == "__main__":
    nc = build_nc(2.0 ** 14, 2.0 ** 11, 2.0 ** 11, 2.0 ** 14)
    print("built ok")
